# revision 54
# baseline (speedup 1.0000x reference)
"""Trainium2 Bass kernel for nn_MultiHeadAttention_81363860455568.

Reference computation (B=2, S=2048, D=1024, H=16, DK=64):
    qh = split_heads(q @ Wq.T); kh, vh likewise
    scores = softmax(qh @ kh.T / 8, axis=-1)
    scores = scores * reaches[:,None,None,:]            (per key)
    scores = scores * (1 - 0.999999*eye(S))             (diagonal suppression)
    out = vh - scores @ vh
    out = out * contrib[:,None,:,None]                  (per query)
    y = concat_heads(out) @ Wo.T

Sharding: 8 cores = 2 batches x 4 head-groups (4 heads each). Each core
receives its batch's transposed activations qT/kT/vT [D, S] in bf16 plus the
head-group slices of Wq/Wk/Wv (as [D, 256]) and Wo (as [256, D]), and returns
a partial y [S, D] (fp32) that the host sums across the 4 head-groups.

Structure (per core):
  - B1: scoresT[k, q] bf16 PSUM ([128, 2048] = both heads of a pair), one
    exp per kb tile -> e in fp8e4m3; diag blocks also make d2neg = e*(-.999999 I).
  - B2 (q-major AV): av[q, 0:65] = sum_k eT[k,q]^T @ [vnat | -1]; col 64
    accumulates -denom for free. A d2neg rider matmul masks the diagonal
    inside the same accumulation (denominator stays unmasked).
  - epilogue: rc = contrib * recip(-denom); cat = (av * rc) + vh*contrib
    (contrib folded into vh at projection time).
  - cat[q, gd] -> catT[gd, q] via XBAR dma transpose; Wo matmuls; y DMA'd
    straight from PSUM.
  - emission interleaves B1(step i) with B2(step i-1) at kb granularity so
    AV/Wo/projection matmuls fill PE gaps under the ACT-bound exp stream.
"""

import functools

import numpy as np
import ml_dtypes

import concourse.bass as bass
import concourse.mybir as mybir
import concourse.tile as tile
from concourse import bacc
from concourse.bass_utils import run_bass_kernel_spmd

BF16 = mybir.dt.bfloat16
F32 = mybir.dt.float32
FP8 = mybir.dt.float8e5
E_DT = FP8
EPOOL_BUFS = 32
TEST_NO_BIAS = False
XPOOL_BUFS = 6

B, S, D, H = 2, 2048, 1024, 16
DK = D // H          # 64
HG = 4               # heads per core (head group)
GD = HG * DK         # 256 head-group dims per core
NKC = D // 128       # 8 contraction chunks for projections
NKB = S // 128       # 16 key blocks
NMS = S // 128       # 16 row blocks
NCC = S // 512       # 4 column chunks of 512 for input staging

DEBUG_OUT = False
Exp = mybir.ActivationFunctionType.Exp
MULT = mybir.AluOpType.mult
ADD = mybir.AluOpType.add


def _emit_kernel(tc: tile.TileContext):
    nc = tc.nc

    qT = nc.declare_dram_parameter("qT", [D, S], BF16, isOutput=False).ap()
    kT = nc.declare_dram_parameter("kT", [D, S], BF16, isOutput=False).ap()
    vT = nc.declare_dram_parameter("vT", [D, S], BF16, isOutput=False).ap()
    wq = nc.declare_dram_parameter("wq", [D, GD], BF16, isOutput=False).ap()
    wk = nc.declare_dram_parameter("wk", [D, GD], BF16, isOutput=False).ap()
    wv = nc.declare_dram_parameter("wv", [D, GD], BF16, isOutput=False).ap()
    wo = nc.declare_dram_parameter("wo", [GD, D], BF16, isOutput=False).ap()
    rcol = nc.declare_dram_parameter("rcol", [128, NKB], F32, isOutput=False).ap()
    ccol = nc.declare_dram_parameter("ccol", [128, NMS], F32, isOutput=False).ap()
    y = nc.declare_dram_parameter("y", [S, D], BF16, isOutput=True).ap()

    # ---------------- resident SBUF buffers ----------------
    consts = tc.alloc_tile_pool(name="consts", bufs=1)
    wq_sb = consts.tile([128, NKC, GD], BF16)
    wk_sb = consts.tile([128, NKC, GD], BF16)
    wv_sb = consts.tile([128, NKC, GD], BF16)
    wo_sb = consts.tile([128, 2, D], BF16)
    rr = consts.tile([128, NKB], F32)
    cc = consts.tile([128, NMS], F32)
    eyeneg = consts.tile([128, 128], E_DT)
    ones_row = consts.tile([1, 2], BF16)
    nbias = consts.tile([128, 1], F32)

    res = tc.alloc_tile_pool(name="res", bufs=1)
    qhT2 = [res.tile([128, S], BF16, name=f"qhT2_{p}") for p in range(2)]
    khT2 = [res.tile([128, S], BF16, name=f"khT2_{p}") for p in range(2)]
    # vh (contrib-scaled) natural [s, gd]
    vhc = res.tile([128, NMS, GD], BF16)
    # reaches-scaled V with a trailing -1 column per head: [k, kb, g, 65]
    vno = res.tile([128, NKB, HG, DK + 1], E_DT)
    catT = [res.tile([128, S], BF16, name=f"catT_{p}") for p in range(2)]
    consts.seal()
    res.seal()

    # constant setup: eyeneg = -0.999999 * I
    nc.gpsimd.memset(eyeneg, 0.0)
    nc.gpsimd.affine_select(
        out=eyeneg, in_=eyeneg,
        compare_op=mybir.AluOpType.not_equal,
        fill=-0.999999, base=0, pattern=[[-1, 128]], channel_multiplier=1,
    )
    nc.gpsimd.memset(ones_row, 1.0)
    nc.gpsimd.memset(nbias, -3.5)
    # -1 sentinel column of vno (denominator accumulator source)
    nc.gpsimd.memset(vno[:, :, :, DK], -1.0)

    # ---------------- pools ----------------
    spsum_cm = tc.tile_pool(name="spsum", bufs=2, space="PSUM")
    spsum = spsum_cm.__enter__()
    work_cm = tc.tile_pool(name="work", bufs=2, space="PSUM")
    work = work_cm.__enter__()
    sbwork_cm = tc.tile_pool(name="sbwork", bufs=1)
    sbwork = sbwork_cm.__enter__()
    epool_cm = tc.tile_pool(name="epool", bufs=EPOOL_BUFS + 6)
    epool = epool_cm.__enter__()
    xpool_cm = tc.tile_pool(name="xpool", bufs=XPOOL_BUFS)
    xpool = xpool_cm.__enter__()

    # Pre-load the exp activation table while ACT is idle so the ~1.3us
    # LoadActFuncSet is off the first real exp's critical path.
    wrm = sbwork.tile([1, 2], F32, tag="wrm", bufs=1)
    nc.scalar.activation(wrm, ones_row[0:1, 0:2], Exp)

    # ---------------- input DMAs (priority order, single queue) ----------
    chunks = {}   # (tensor, c) -> [128, NKC, 512] staging tile

    def dma_in_chunk(tname, src, c):
        ch = xpool.tile([128, NKC, 512], BF16, tag="xch", name=f"x_{tname}{c}")
        nc.sync.dma_start(
            out=ch,
            in_=src[:, c * 512:(c + 1) * 512].rearrange("(a p) s -> p a s", p=128),
        )
        chunks[(tname, c)] = ch

    def dma_in_w(dst_sb, src):
        nc.sync.dma_start(
            out=dst_sb, in_=src.rearrange("(a p) s -> p a s", p=128))

    dma_in_w(wk_sb, wk)
    dma_in_chunk("k", kT, 0)
    dma_in_w(wq_sb, wq)
    dma_in_chunk("q", qT, 0)
    dma_in_chunk("q", qT, 1)
    for c in range(1, NCC):
        dma_in_chunk("k", kT, c)
    dma_in_w(wv_sb, wv)
    nc.sync.dma_start(out=rr, in_=rcol)
    nc.sync.dma_start(out=cc, in_=ccol)
    for c in range(NCC):
        dma_in_chunk("v", vT, c)
    for c in range(2, NCC):
        dma_in_chunk("q", qT, c)
    dma_in_w(wo_sb, wo)

    # --- projection units (emitted lazily as interleave filler) ---
    def proj_qk_unit(dst2, w_sb, tname, p, nq):
        # one [128, 512] column chunk of qhT/khT pair p
        ps = work.tile([128, 512], F32, tag="pw", name="ps")
        ch = chunks[(tname, nq)]
        for kc in range(NKC):
            nc.tensor.matmul(
                ps,
                lhsT=w_sb[:, kc, p * 128:(p + 1) * 128],
                rhs=ch[:, kc, :],
                start=(kc == 0), stop=(kc == NKC - 1),
            )
        nc.vector.tensor_copy(dst2[p][:, nq * 512:(nq + 1) * 512], ps)

    def proj_v_unit(ms):
        ps = work.tile([128, 512], F32, tag="pw", name="ps")
        ch = chunks[("v", ms // 4)]
        for kc in range(NKC):
            nc.tensor.matmul(
                ps[:, :GD],
                lhsT=ch[:, kc, (ms % 4) * 128:(ms % 4 + 1) * 128],
                rhs=wv_sb[:, kc, :],
                start=(kc == 0), stop=(kc == NKC - 1),
            )
        # contrib-scaled vh (epilogue in1) and reach-scaled vno (AV rhs)
        nc.vector.tensor_scalar_mul(vhc[:, ms, :], ps[:, :GD], cc[:, ms:ms + 1])
        for g in range(HG):
            nc.vector.tensor_scalar_mul(
                vno[:, ms, g, 0:DK], ps[:, g * DK:(g + 1) * DK], rr[:, ms:ms + 1])

    # ---------------- B1: scores + exp (+ d2neg on diag blocks) ----------
    d2negs = {}   # (p, h, kb) -> [128, 128] fp8 tile (diag blocks only)
    etiles = {}   # (p, kb) -> [128, 2*1024] fp8 (cols h*1024 + q_local)

    def emit_b1_kb_halfq(half, p, kb, qc, et):
        # early variant: one 512-wide q chunk of a kb tile, both heads,
        # through [128,512] av-tag psum (sp tiles stay free for the stream)
        q0 = half * 1024
        for h in range(2):
            r0, r1 = h * 64, h * 64 + 64
            spq = work.tile([128, 512], F32, tag="av", name="spq")
            nc.tensor.matmul(
                spq,
                lhsT=khT2[p][r0:r1, kb * 128:(kb + 1) * 128],
                rhs=qhT2[p][r0:r1, q0 + qc * 512:q0 + (qc + 1) * 512],
                start=True, stop=True,
                tile_position=(h * 64, 0),
            )
            nc.scalar.activation(
                et[:, h * 1024 + qc * 512:h * 1024 + (qc + 1) * 512],
                spq, Exp, scale=0.125)

    def emit_b1_kb(half, p, kb):
        q0 = half * 1024
        et = epool.tile([128, 2048], E_DT, tag="e", name="et")
        for h in range(2):
            sp = spsum.tile([128, 1024], F32, tag="sc", name="sp")
            r0, r1 = h * 64, h * 64 + 64
            for qc in range(2):
                nc.tensor.matmul(
                    sp[:, qc * 512:(qc + 1) * 512],
                    lhsT=khT2[p][r0:r1, kb * 128:(kb + 1) * 128],
                    rhs=qhT2[p][r0:r1, q0 + qc * 512:q0 + (qc + 1) * 512],
                    start=True, stop=True,
                    tile_position=(h * 64, 0),
                )
            # global -2 shift keeps e = exp(s/8 - 2) inside fp8e4m3 range;
            # softmax is invariant to a uniform shift (denominator scales too)
            nc.scalar.activation(
                et[:, h * 1024:(h + 1) * 1024], sp, Exp, scale=0.125)
        etiles[(p, kb)] = et

    # cat natural tiles, one per (half, q subtile); double-buffered so the
    # second half's epilogue never races the first half's transpose DMA
    catn_tiles = {}

    def catn_for(half, qs):
        key = (half, qs)
        if key not in catn_tiles:
            catn_tiles[key] = sbwork.tile(
                [128, 256], BF16, tag=f"catn{qs}", bufs=2, name=f"catn{qs}")
        return catn_tiles[key]


    def emit_d2_kb(half, p, kb):
        # d2neg for one diagonal block, right after its exp so the DVE queue
        # stays time-aligned (no backlog ahead of epilogue ops)
        qg = kb
        off = 128 * (kb - 8 * half)
        for h in range(2):
            d2 = sbwork.tile([128, 128], E_DT, tag="d2", bufs=32, name="d2")
            nc.vector.tensor_mul(
                d2, etiles[(p, kb)][:, h * 1024 + off:h * 1024 + off + 128],
                eyeneg)
            d2negs[(p, h, qg)] = d2

    # ---------------- B2: q-major AV + epilogue ----------------
    av_tiles = {}

    def emit_av_mm(half, p, qs, h):
        # one 128-query subtile, one head of pair p
        qg = half * 8 + qs           # global 128-block index == diag kb
        key = (half, p, qs)
        if key not in av_tiles:
            av_tiles[key] = work.tile([128, 512], F32, tag="av", name="av")
        av = av_tiles[key]
        g = p * 2 + h
        c0 = h * (DK + 1)
        for kb in range(NKB):
            if kb == NKB - 1:
                # diagonal suppression rider, inside the accumulation group
                # (skips the -denom column: denominator stays unmasked)
                nc.tensor.matmul(
                    av[:, c0:c0 + DK],
                    lhsT=d2negs[(p, h, qg)],
                    rhs=vno[:, qg, g, 0:DK],
                    start=False, stop=False,
                    skip_group_check=True,
                )
            nc.tensor.matmul(
                av[:, c0:c0 + DK + 1],
                lhsT=etiles[(p, kb)][
                    :, h * 1024 + qs * 128:h * 1024 + (qs + 1) * 128],
                rhs=vno[:, kb, g, :],
                start=(kb == 0), stop=(kb == NKB - 1),
                skip_group_check=True,
            )

    def emit_av_epi(half, p, qs, h):
        qg = half * 8 + qs
        av = av_tiles[(half, p, qs)]
        g = p * 2 + h
        c0 = h * (DK + 1)
        rn = sbwork.tile([128, 1], F32, tag="rn", bufs=8, name="rn")
        nc.vector.reciprocal(rn, av[:, c0 + DK:c0 + DK + 1])
        rc = sbwork.tile([128, 1], F32, tag="rc", bufs=8, name="rc")
        nc.vector.tensor_mul(rc, rn, cc[:, qg:qg + 1])
        # cat = (av * (-contrib/denom)) + vh*contrib
        nc.vector.scalar_tensor_tensor(
            out=catn_for(half, qs)[:, p * 128 + h * 64:p * 128 + (h + 1) * 64],
            in0=av[:, c0:c0 + DK],
            scalar=rc,
            in1=vhc[:, qg, g * DK:(g + 1) * DK],
            op0=MULT, op1=ADD,
        )

    def emit_transposes(half, qs):
        qg = half * 8 + qs
        for pp in range(2):
            nc.sync.dma_start_transpose(
                out=catT[pp][:, qg * 128:(qg + 1) * 128],
                in_=catn_for(half, qs)[:, pp * 128:(pp + 1) * 128],
            )

    def emit_fin_oc(half, qs, oc, tail=False, transpose=True, _ysbs={}):
        qg = half * 8 + qs
        if oc == 0 and transpose:
            emit_transposes(half, qs)
        wop = work.tile([128, 512], F32, tag="pw", name="wop")
        for pp in range(2):
            nc.tensor.matmul(
                wop,
                lhsT=catT[pp][:, qg * 128:(qg + 1) * 128],
                rhs=wo_sb[:, pp, oc * 512:(oc + 1) * 512],
                start=(pp == 0), stop=(pp == 1),
            )
        if oc == 0:
            _ysbs[qg] = sbwork.tile(
                [128, 1024], BF16, tag="ysb", bufs=4, name="y_sb")
        y_sb = _ysbs[qg]
        if tail:
            nc.scalar.activation(
                y_sb[:, oc * 512:(oc + 1) * 512], wop,
                mybir.ActivationFunctionType.Copy)
        else:
            nc.vector.tensor_copy(y_sb[:, oc * 512:(oc + 1) * 512], wop)
        if oc == 1:
            # one merged DMA per q block (halves HWDGE/SP issue count)
            nc.sync.dma_start(
                out=y[qg * 128:(qg + 1) * 128, :], in_=y_sb)

    # ---------------- schedule ----------------
    # lead-in: K p0 chunk 0 + Q p0 chunk 0, then half-width scores for
    # kb0/kb1 so the exp stream starts before qT chunk 1 has landed
    proj_qk_unit(khT2, wk_sb, "k", 0, 0)
    proj_qk_unit(qhT2, wq_sb, "q", 0, 0)
    early_ets = {}
    for kb in range(2):
        early_ets[kb] = epool.tile([128, 2048], E_DT, tag="e", name="et")
        emit_b1_kb_halfq(0, 0, kb, 0, early_ets[kb])
    proj_qk_unit(qhT2, wq_sb, "q", 0, 1)
    for kb in range(2):
        emit_b1_kb_halfq(0, 0, kb, 1, early_ets[kb])
        etiles[(0, kb)] = early_ets[kb]

    s0_fill = []
    for nq in range(1, 4):
        s0_fill.append(lambda nq=nq: proj_qk_unit(khT2, wk_sb, "k", 0, nq))
    for nq in range(4):
        s0_fill.append(lambda nq=nq: proj_qk_unit(khT2, wk_sb, "k", 1, nq))
    for nq in range(2):
        s0_fill.append(lambda nq=nq: proj_qk_unit(qhT2, wq_sb, "q", 1, nq))
    for ms in range(NMS):
        s0_fill.append(lambda ms=ms: proj_v_unit(ms))

    s1_fill = []
    for nq in range(2, 4):
        s1_fill.append(lambda nq=nq: proj_qk_unit(qhT2, wq_sb, "q", 0, nq))
        s1_fill.append(lambda nq=nq: proj_qk_unit(qhT2, wq_sb, "q", 1, nq))

    steps = [(0, 0), (0, 1), (1, 0), (1, 1)]

    def b2_units(i, tail=False, split_fins=False):
        half, p = steps[i]
        avs, fins = [], []
        for qs in range(8):
            def av_unit(half=half, p=p, qs=qs):
                emit_av_mm(half, p, qs, 0)
                emit_av_mm(half, p, qs, 1)
                emit_av_epi(half, p, qs, 0)
                emit_av_epi(half, p, qs, 1)
            avs.append(av_unit)
            if p == 1:
                def fin_unit(half=half, qs=qs, tail=tail):
                    emit_fin_oc(half, qs, 0, tail)
                    emit_fin_oc(half, qs, 1, tail)
                fins.append(fin_unit)
        if not fins:
            return avs
        if split_fins:
            # avs + first half of fins interleaved; rest deferred to the
            # next step's filler list (balances PE load across B1 windows)
            units = []
            for qs in range(8):
                units.append(avs[qs])
                if qs % 2 == 0:
                    units.append(fins[qs // 2])
            rest = fins[4:]
            return units, rest
        if tail:
            # tail: all AV work first, then all transposes, then the Wo/y
            # chain -- PE never blocks mid-stream on a transpose DMA
            def all_transposes(half=half):
                for qs in range(8):
                    emit_transposes(half, qs)
            nofins = []
            for qs in range(8):
                def fin_unit(half=half, qs=qs):
                    emit_fin_oc(half, qs, 0, tail=True, transpose=False)
                    emit_fin_oc(half, qs, 1, tail=True, transpose=False)
                nofins.append(fin_unit)
            return avs + [all_transposes] + nofins
        units = []
        for qs in range(8):
            units.append(avs[qs])
            units.append(fins[qs])
        return units

    for i, (half, p) in enumerate(steps):
        if i == 0:
            pending = s0_fill
        elif i == 1:
            pending = s1_fill + b2_units(0)
        else:
            pending = b2_units(i - 1)
        nfill = len(pending)
        done = 0
        for kb in range(NKB):
            if i == 0 and kb < 2:
                pass   # emitted in the lead-in (half-width variant)
            else:
                emit_b1_kb(half, p, kb)
            if 8 * half <= kb < 8 * half + 8:
                emit_d2_kb(half, p, kb)
            want = (nfill * (kb + 1) + NKB - 1) // NKB
            while done < want:
                pending[done]()
                done += 1
    for u in b2_units(3, tail=True):
        u()

    xpool_cm.__exit__(None, None, None)
    epool_cm.__exit__(None, None, None)
    sbwork_cm.__exit__(None, None, None)
    work_cm.__exit__(None, None, None)
    spsum_cm.__exit__(None, None, None)


@functools.cache
def build_nc() -> bass.Bass:
    nc = bacc.Bacc("TRN2", target_bir_lowering=False, debug=False)
    with tile.TileContext(nc) as tc:
        _emit_kernel(tc)
    nc.compile()
    return nc


def _prep_inputs(q, k, v, reaches, Wq, Wk, Wv, Wo):
    """Host-side shard + layout prep. Returns per-core input maps."""
    bf16 = ml_dtypes.bfloat16
    r = np.asarray(reaches, np.float32)
    rs = r.sum(axis=-1, keepdims=True)
    contrib = (rs - r) / (rs + 1e-9) * (1.0 - r) * 100.0  # [B, S] f32

    per_batch = []
    for b in range(B):
        qTb = np.ascontiguousarray(np.asarray(q[b], np.float32).T.astype(bf16))
        kTb = np.ascontiguousarray(np.asarray(k[b], np.float32).T.astype(bf16))
        vTb = np.ascontiguousarray(np.asarray(v[b], np.float32).T.astype(bf16))
        # [128, NKB] with [p, c] = vec[128*c + p]
        rcol = np.ascontiguousarray(r[b].reshape(NKB, 128).T)
        ccol = np.ascontiguousarray(contrib[b].reshape(NMS, 128).T)
        per_batch.append((qTb, kTb, vTb, rcol, ccol))

    in_maps = []
    for c in range(8):
        b, g = divmod(c, 4)
        hs = slice(g * GD, (g + 1) * GD)
        qTb, kTb, vTb, rcol, ccol = per_batch[b]
        in_maps.append({
            "qT": qTb, "kT": kTb, "vT": vTb,
            "wq": np.ascontiguousarray(np.asarray(Wq, np.float32)[hs, :].T).astype(bf16),
            "wk": np.ascontiguousarray(np.asarray(Wk, np.float32)[hs, :].T).astype(bf16),
            "wv": np.ascontiguousarray(np.asarray(Wv, np.float32)[hs, :].T).astype(bf16),
            "wo": np.ascontiguousarray(np.asarray(Wo, np.float32)[:, hs].T).astype(bf16),
            "rcol": rcol, "ccol": ccol,
        })
    return in_maps


def kernel(q, k, v, reaches, Wq, Wk, Wv, Wo, **run_kwargs):
    nc = build_nc()
    in_maps = _prep_inputs(q, k, v, reaches, Wq, Wk, Wv, Wo)
    res = run_bass_kernel_spmd(nc, in_maps, list(range(8)), **run_kwargs)
    out = np.zeros((B, S, D), np.float32)
    for c in range(8):
        b = c // 4
        out[b] += np.asarray(res.results[c]["y"], dtype=np.float32)
    kernel.last_results = res
    return out


# revision 55
# speedup vs baseline: 1.0203x; 1.0203x over previous
"""Trainium2 Bass kernel for nn_MultiHeadAttention_81363860455568.

Reference computation (B=2, S=2048, D=1024, H=16, DK=64):
    qh = split_heads(q @ Wq.T); kh, vh likewise
    scores = softmax(qh @ kh.T / 8, axis=-1)
    scores = scores * reaches[:,None,None,:]            (per key)
    scores = scores * (1 - 0.999999*eye(S))             (diagonal suppression)
    out = vh - scores @ vh
    out = out * contrib[:,None,:,None]                  (per query)
    y = concat_heads(out) @ Wo.T

Sharding: 8 cores = 2 batches x 4 head-groups (4 heads each). Each core
receives its batch's transposed activations qT/kT/vT [D, S] in bf16 plus the
head-group slices of Wq/Wk/Wv (as [D, 256]) and Wo (as [256, D]), and returns
a partial y [S, D] (fp32) that the host sums across the 4 head-groups.

Structure (per core):
  - B1: scoresT[k, q] bf16 PSUM ([128, 2048] = both heads of a pair), one
    exp per kb tile -> e in fp8e4m3; diag blocks also make d2neg = e*(-.999999 I).
  - B2 (q-major AV): av[q, 0:65] = sum_k eT[k,q]^T @ [vnat | -1]; col 64
    accumulates -denom for free. A d2neg rider matmul masks the diagonal
    inside the same accumulation (denominator stays unmasked).
  - epilogue: rc = contrib * recip(-denom); cat = (av * rc) + vh*contrib
    (contrib folded into vh at projection time).
  - cat[q, gd] -> catT[gd, q] via XBAR dma transpose; Wo matmuls; y DMA'd
    straight from PSUM.
  - emission interleaves B1(step i) with B2(step i-1) at kb granularity so
    AV/Wo/projection matmuls fill PE gaps under the ACT-bound exp stream.
"""

import functools

import numpy as np
import ml_dtypes

import concourse.bass as bass
import concourse.mybir as mybir
import concourse.tile as tile
from concourse import bacc
from concourse.bass_utils import run_bass_kernel_spmd

BF16 = mybir.dt.bfloat16
F32 = mybir.dt.float32
FP8 = mybir.dt.float8e5
E_DT = FP8
EPOOL_BUFS = 32
TEST_NO_BIAS = False
XPOOL_BUFS = 6

B, S, D, H = 2, 2048, 1024, 16
DK = D // H          # 64
HG = 4               # heads per core (head group)
GD = HG * DK         # 256 head-group dims per core
NKC = D // 128       # 8 contraction chunks for projections
NKB = S // 128       # 16 key blocks
NMS = S // 128       # 16 row blocks
NCC = S // 512       # 4 column chunks of 512 for input staging

DEBUG_OUT = False
Exp = mybir.ActivationFunctionType.Exp
MULT = mybir.AluOpType.mult
ADD = mybir.AluOpType.add


def _emit_kernel(tc: tile.TileContext):
    nc = tc.nc

    qT = nc.declare_dram_parameter("qT", [D, S], BF16, isOutput=False).ap()
    kT = nc.declare_dram_parameter("kT", [D, S], BF16, isOutput=False).ap()
    vT = nc.declare_dram_parameter("vT", [D, S], BF16, isOutput=False).ap()
    wq = nc.declare_dram_parameter("wq", [D, GD], BF16, isOutput=False).ap()
    wk = nc.declare_dram_parameter("wk", [D, GD], BF16, isOutput=False).ap()
    wv = nc.declare_dram_parameter("wv", [D, GD], BF16, isOutput=False).ap()
    wo = nc.declare_dram_parameter("wo", [GD, D], BF16, isOutput=False).ap()
    rcol = nc.declare_dram_parameter("rcol", [128, NKB], F32, isOutput=False).ap()
    ccol = nc.declare_dram_parameter("ccol", [128, NMS], F32, isOutput=False).ap()
    y = nc.declare_dram_parameter("y", [S, D], BF16, isOutput=True).ap()

    # ---------------- resident SBUF buffers ----------------
    consts = tc.alloc_tile_pool(name="consts", bufs=1)
    wq_sb = consts.tile([128, NKC, GD], BF16)
    wk_sb = consts.tile([128, NKC, GD], BF16)
    wv_sb = consts.tile([128, NKC, GD], BF16)
    wo_sb = consts.tile([128, 2, D], BF16)
    rr = consts.tile([128, NKB], F32)
    cc = consts.tile([128, NMS], F32)
    eyeneg = consts.tile([128, 128], E_DT)
    ones_row = consts.tile([1, 2], BF16)
    nbias = consts.tile([128, 1], F32)

    res = tc.alloc_tile_pool(name="res", bufs=1)
    qhT2 = [res.tile([128, S], BF16, name=f"qhT2_{p}") for p in range(2)]
    khT2 = [res.tile([128, S], BF16, name=f"khT2_{p}") for p in range(2)]
    # vh (contrib-scaled) natural [s, gd]
    vhc = res.tile([128, NMS, GD], BF16)
    # reaches-scaled V with a trailing -1 column per head: [k, kb, g, 65]
    vno = res.tile([128, NKB, HG, DK + 1], E_DT)
    catT = [res.tile([128, S], BF16, name=f"catT_{p}") for p in range(2)]
    consts.seal()
    res.seal()

    # constant setup: eyeneg = -0.999999 * I
    nc.gpsimd.memset(eyeneg, 0.0)
    nc.gpsimd.affine_select(
        out=eyeneg, in_=eyeneg,
        compare_op=mybir.AluOpType.not_equal,
        fill=-0.999999, base=0, pattern=[[-1, 128]], channel_multiplier=1,
    )
    nc.gpsimd.memset(ones_row, 1.0)
    nc.gpsimd.memset(nbias, -3.5)
    # -1 sentinel column of vno (denominator accumulator source)
    nc.gpsimd.memset(vno[:, :, :, DK], -1.0)

    # ---------------- pools ----------------
    spsum_cm = tc.tile_pool(name="spsum", bufs=2, space="PSUM")
    spsum = spsum_cm.__enter__()
    work_cm = tc.tile_pool(name="work", bufs=2, space="PSUM")
    work = work_cm.__enter__()
    sbwork_cm = tc.tile_pool(name="sbwork", bufs=1)
    sbwork = sbwork_cm.__enter__()
    epool_cm = tc.tile_pool(name="epool", bufs=EPOOL_BUFS + 8)
    epool = epool_cm.__enter__()
    xpool_cm = tc.tile_pool(name="xpool", bufs=XPOOL_BUFS)
    xpool = xpool_cm.__enter__()

    # Pre-load the exp activation table while ACT is idle so the ~1.3us
    # LoadActFuncSet is off the first real exp's critical path.
    wrm = sbwork.tile([1, 2], F32, tag="wrm", bufs=1)
    nc.scalar.activation(wrm, ones_row[0:1, 0:2], Exp)

    # ---------------- input DMAs (priority order, single queue) ----------
    chunks = {}   # (tensor, c) -> [128, NKC, 512] staging tile

    def dma_in_chunk(tname, src, c):
        ch = xpool.tile([128, NKC, 512], BF16, tag="xch", name=f"x_{tname}{c}")
        nc.sync.dma_start(
            out=ch,
            in_=src[:, c * 512:(c + 1) * 512].rearrange("(a p) s -> p a s", p=128),
        )
        chunks[(tname, c)] = ch

    def dma_in_w(dst_sb, src):
        nc.sync.dma_start(
            out=dst_sb, in_=src.rearrange("(a p) s -> p a s", p=128))

    dma_in_w(wk_sb, wk)
    dma_in_chunk("k", kT, 0)
    dma_in_w(wq_sb, wq)
    dma_in_chunk("q", qT, 0)
    dma_in_chunk("q", qT, 1)
    for c in range(1, NCC):
        dma_in_chunk("k", kT, c)
    dma_in_w(wv_sb, wv)
    nc.sync.dma_start(out=rr, in_=rcol)
    nc.sync.dma_start(out=cc, in_=ccol)
    for c in range(NCC):
        dma_in_chunk("v", vT, c)
    for c in range(2, NCC):
        dma_in_chunk("q", qT, c)
    dma_in_w(wo_sb, wo)

    # --- projection units (emitted lazily as interleave filler) ---
    def proj_qk_unit(dst2, w_sb, tname, p, nq):
        # one [128, 512] column chunk of qhT/khT pair p
        ps = work.tile([128, 512], F32, tag="pw", name="ps")
        ch = chunks[(tname, nq)]
        for kc in range(NKC):
            nc.tensor.matmul(
                ps,
                lhsT=w_sb[:, kc, p * 128:(p + 1) * 128],
                rhs=ch[:, kc, :],
                start=(kc == 0), stop=(kc == NKC - 1),
            )
        nc.vector.tensor_copy(dst2[p][:, nq * 512:(nq + 1) * 512], ps)

    def proj_v_unit(ms):
        ps = work.tile([128, 512], F32, tag="pw", name="ps")
        ch = chunks[("v", ms // 4)]
        for kc in range(NKC):
            nc.tensor.matmul(
                ps[:, :GD],
                lhsT=ch[:, kc, (ms % 4) * 128:(ms % 4 + 1) * 128],
                rhs=wv_sb[:, kc, :],
                start=(kc == 0), stop=(kc == NKC - 1),
            )
        # contrib-scaled vh (epilogue in1) and reach-scaled vno (AV rhs)
        nc.vector.tensor_scalar_mul(vhc[:, ms, :], ps[:, :GD], cc[:, ms:ms + 1])
        for g in range(HG):
            nc.vector.tensor_scalar_mul(
                vno[:, ms, g, 0:DK], ps[:, g * DK:(g + 1) * DK], rr[:, ms:ms + 1])

    # ---------------- B1: scores + exp (+ d2neg on diag blocks) ----------
    d2negs = {}   # (p, h, kb) -> [128, 128] fp8 tile (diag blocks only)
    etiles = {}   # (p, kb) -> [128, 2*1024] fp8 (cols h*1024 + q_local)

    def emit_b1_kb_halfq(half, p, kb, qc, et):
        # early variant: one 512-wide q chunk of a kb tile, both heads,
        # through [128,512] av-tag psum (sp tiles stay free for the stream)
        q0 = half * 1024
        for h in range(2):
            r0, r1 = h * 64, h * 64 + 64
            spq = work.tile([128, 512], F32, tag="av", name="spq")
            nc.tensor.matmul(
                spq,
                lhsT=khT2[p][r0:r1, kb * 128:(kb + 1) * 128],
                rhs=qhT2[p][r0:r1, q0 + qc * 512:q0 + (qc + 1) * 512],
                start=True, stop=True,
                tile_position=(h * 64, 0),
            )
            nc.scalar.activation(
                et[:, h * 1024 + qc * 512:h * 1024 + (qc + 1) * 512],
                spq, Exp, scale=0.125)

    def emit_b1_kb(half, p, kb):
        q0 = half * 1024
        et = epool.tile([128, 2048], E_DT, tag="e", name="et")
        for h in range(2):
            sp = spsum.tile([128, 1024], F32, tag="sc", name="sp")
            r0, r1 = h * 64, h * 64 + 64
            for qc in range(2):
                nc.tensor.matmul(
                    sp[:, qc * 512:(qc + 1) * 512],
                    lhsT=khT2[p][r0:r1, kb * 128:(kb + 1) * 128],
                    rhs=qhT2[p][r0:r1, q0 + qc * 512:q0 + (qc + 1) * 512],
                    start=True, stop=True,
                    tile_position=(h * 64, 0),
                )
            # global -2 shift keeps e = exp(s/8 - 2) inside fp8e4m3 range;
            # softmax is invariant to a uniform shift (denominator scales too)
            nc.scalar.activation(
                et[:, h * 1024:(h + 1) * 1024], sp, Exp, scale=0.125)
        etiles[(p, kb)] = et

    # cat natural tiles, one per (half, q subtile); double-buffered so the
    # second half's epilogue never races the first half's transpose DMA
    catn_tiles = {}

    def catn_for(half, qs):
        key = (half, qs)
        if key not in catn_tiles:
            catn_tiles[key] = sbwork.tile(
                [128, 256], BF16, tag=f"catn{qs}", bufs=2, name=f"catn{qs}")
        return catn_tiles[key]


    def emit_d2_kb(half, p, kb):
        # d2neg for one diagonal block, right after its exp so the DVE queue
        # stays time-aligned (no backlog ahead of epilogue ops)
        qg = kb
        off = 128 * (kb - 8 * half)
        for h in range(2):
            d2 = sbwork.tile([128, 128], E_DT, tag="d2", bufs=32, name="d2")
            nc.vector.tensor_mul(
                d2, etiles[(p, kb)][:, h * 1024 + off:h * 1024 + off + 128],
                eyeneg)
            d2negs[(p, h, qg)] = d2

    # ---------------- B2: q-major AV + epilogue ----------------
    av_tiles = {}

    def emit_av_mm(half, p, qs, h):
        # one 128-query subtile, one head of pair p
        qg = half * 8 + qs           # global 128-block index == diag kb
        key = (half, p, qs)
        if key not in av_tiles:
            av_tiles[key] = work.tile([128, 512], F32, tag="av", name="av")
        av = av_tiles[key]
        g = p * 2 + h
        c0 = h * (DK + 1)
        for kb in range(NKB):
            if kb == NKB - 1:
                # diagonal suppression rider, inside the accumulation group
                # (skips the -denom column: denominator stays unmasked)
                nc.tensor.matmul(
                    av[:, c0:c0 + DK],
                    lhsT=d2negs[(p, h, qg)],
                    rhs=vno[:, qg, g, 0:DK],
                    start=False, stop=False,
                    skip_group_check=True,
                )
            nc.tensor.matmul(
                av[:, c0:c0 + DK + 1],
                lhsT=etiles[(p, kb)][
                    :, h * 1024 + qs * 128:h * 1024 + (qs + 1) * 128],
                rhs=vno[:, kb, g, :],
                start=(kb == 0), stop=(kb == NKB - 1),
                skip_group_check=True,
            )

    def emit_av_epi(half, p, qs, h):
        qg = half * 8 + qs
        av = av_tiles[(half, p, qs)]
        g = p * 2 + h
        c0 = h * (DK + 1)
        rn = sbwork.tile([128, 1], F32, tag="rn", bufs=12, name="rn")
        nc.vector.reciprocal(rn, av[:, c0 + DK:c0 + DK + 1])
        rc = sbwork.tile([128, 1], F32, tag="rc", bufs=12, name="rc")
        nc.vector.tensor_mul(rc, rn, cc[:, qg:qg + 1])
        # cat = (av * (-contrib/denom)) + vh*contrib
        nc.vector.scalar_tensor_tensor(
            out=catn_for(half, qs)[:, p * 128 + h * 64:p * 128 + (h + 1) * 64],
            in0=av[:, c0:c0 + DK],
            scalar=rc,
            in1=vhc[:, qg, g * DK:(g + 1) * DK],
            op0=MULT, op1=ADD,
        )

    def emit_transposes(half, qs):
        qg = half * 8 + qs
        for pp in range(2):
            nc.sync.dma_start_transpose(
                out=catT[pp][:, qg * 128:(qg + 1) * 128],
                in_=catn_for(half, qs)[:, pp * 128:(pp + 1) * 128],
            )

    def emit_fin_oc(half, qs, oc, tail=False, transpose=True, _ysbs={}):
        qg = half * 8 + qs
        if oc == 0 and transpose:
            emit_transposes(half, qs)
        wop = work.tile([128, 512], F32, tag="pw", name="wop")
        for pp in range(2):
            nc.tensor.matmul(
                wop,
                lhsT=catT[pp][:, qg * 128:(qg + 1) * 128],
                rhs=wo_sb[:, pp, oc * 512:(oc + 1) * 512],
                start=(pp == 0), stop=(pp == 1),
            )
        if oc == 0:
            _ysbs[qg] = sbwork.tile(
                [128, 1024], BF16, tag="ysb", bufs=6, name="y_sb")
        y_sb = _ysbs[qg]
        if tail:
            nc.scalar.activation(
                y_sb[:, oc * 512:(oc + 1) * 512], wop,
                mybir.ActivationFunctionType.Copy)
        else:
            nc.vector.tensor_copy(y_sb[:, oc * 512:(oc + 1) * 512], wop)
        if oc == 1:
            # one merged DMA per q block (halves HWDGE/SP issue count)
            nc.sync.dma_start(
                out=y[qg * 128:(qg + 1) * 128, :], in_=y_sb)

    # ---------------- schedule ----------------
    # lead-in: K p0 chunk 0 + Q p0 chunk 0, then half-width scores for
    # kb0/kb1 so the exp stream starts before qT chunk 1 has landed
    proj_qk_unit(khT2, wk_sb, "k", 0, 0)
    proj_qk_unit(qhT2, wq_sb, "q", 0, 0)
    early_ets = {}
    for kb in range(2):
        early_ets[kb] = epool.tile([128, 2048], E_DT, tag="e", name="et")
        emit_b1_kb_halfq(0, 0, kb, 0, early_ets[kb])
    proj_qk_unit(qhT2, wq_sb, "q", 0, 1)
    for kb in range(2):
        emit_b1_kb_halfq(0, 0, kb, 1, early_ets[kb])
        etiles[(0, kb)] = early_ets[kb]

    s0_fill = []
    for nq in range(1, 4):
        s0_fill.append(lambda nq=nq: proj_qk_unit(khT2, wk_sb, "k", 0, nq))
    for nq in range(4):
        s0_fill.append(lambda nq=nq: proj_qk_unit(khT2, wk_sb, "k", 1, nq))
    for nq in range(2):
        s0_fill.append(lambda nq=nq: proj_qk_unit(qhT2, wq_sb, "q", 1, nq))
    for ms in range(NMS):
        s0_fill.append(lambda ms=ms: proj_v_unit(ms))

    s1_fill = []
    for nq in range(2, 4):
        s1_fill.append(lambda nq=nq: proj_qk_unit(qhT2, wq_sb, "q", 0, nq))
        s1_fill.append(lambda nq=nq: proj_qk_unit(qhT2, wq_sb, "q", 1, nq))

    steps = [(0, 0), (0, 1), (1, 0), (1, 1)]

    def b2_units(i, tail=False, split_fins=False):
        half, p = steps[i]
        avs, fins = [], []
        for qs in range(8):
            def av_unit(half=half, p=p, qs=qs):
                emit_av_mm(half, p, qs, 0)
                emit_av_mm(half, p, qs, 1)
                emit_av_epi(half, p, qs, 0)
                emit_av_epi(half, p, qs, 1)
            avs.append(av_unit)
            if p == 1:
                def fin_unit(half=half, qs=qs, tail=tail):
                    emit_fin_oc(half, qs, 0, tail)
                    emit_fin_oc(half, qs, 1, tail)
                fins.append(fin_unit)
        if not fins:
            return avs
        if split_fins:
            # avs + first half of fins interleaved; rest deferred to the
            # next step's filler list (balances PE load across B1 windows)
            units = []
            for qs in range(8):
                units.append(avs[qs])
                if qs % 2 == 0:
                    units.append(fins[qs // 2])
            rest = fins[4:]
            return units, rest
        if tail:
            # tail: all AV work first, then all transposes, then the Wo/y
            # chain -- PE never blocks mid-stream on a transpose DMA
            def all_transposes(half=half):
                for qs in range(8):
                    emit_transposes(half, qs)
            nofins = []
            for qs in range(8):
                def fin_unit(half=half, qs=qs):
                    emit_fin_oc(half, qs, 0, tail=True, transpose=False)
                    emit_fin_oc(half, qs, 1, tail=True, transpose=False)
                nofins.append(fin_unit)
            return avs + [all_transposes] + nofins
        units = []
        for qs in range(8):
            units.append(avs[qs])
            units.append(fins[qs])
        return units

    for i, (half, p) in enumerate(steps):
        if i == 0:
            pending = s0_fill
        elif i == 1:
            pending = s1_fill + b2_units(0)
        else:
            pending = b2_units(i - 1)
        nfill = len(pending)
        done = 0
        for kb in range(NKB):
            if i == 0 and kb < 2:
                pass   # emitted in the lead-in (half-width variant)
            else:
                emit_b1_kb(half, p, kb)
            if 8 * half <= kb < 8 * half + 8:
                emit_d2_kb(half, p, kb)
            want = (nfill * (kb + 1) + NKB - 1) // NKB
            while done < want:
                pending[done]()
                done += 1
    for u in b2_units(3, tail=True):
        u()

    xpool_cm.__exit__(None, None, None)
    epool_cm.__exit__(None, None, None)
    sbwork_cm.__exit__(None, None, None)
    work_cm.__exit__(None, None, None)
    spsum_cm.__exit__(None, None, None)


@functools.cache
def build_nc() -> bass.Bass:
    nc = bacc.Bacc("TRN2", target_bir_lowering=False, debug=False)
    with tile.TileContext(nc) as tc:
        _emit_kernel(tc)
    nc.compile()
    return nc


def _prep_inputs(q, k, v, reaches, Wq, Wk, Wv, Wo):
    """Host-side shard + layout prep. Returns per-core input maps."""
    bf16 = ml_dtypes.bfloat16
    r = np.asarray(reaches, np.float32)
    rs = r.sum(axis=-1, keepdims=True)
    contrib = (rs - r) / (rs + 1e-9) * (1.0 - r) * 100.0  # [B, S] f32

    per_batch = []
    for b in range(B):
        qTb = np.ascontiguousarray(np.asarray(q[b], np.float32).T.astype(bf16))
        kTb = np.ascontiguousarray(np.asarray(k[b], np.float32).T.astype(bf16))
        vTb = np.ascontiguousarray(np.asarray(v[b], np.float32).T.astype(bf16))
        # [128, NKB] with [p, c] = vec[128*c + p]
        rcol = np.ascontiguousarray(r[b].reshape(NKB, 128).T)
        ccol = np.ascontiguousarray(contrib[b].reshape(NMS, 128).T)
        per_batch.append((qTb, kTb, vTb, rcol, ccol))

    in_maps = []
    for c in range(8):
        b, g = divmod(c, 4)
        hs = slice(g * GD, (g + 1) * GD)
        qTb, kTb, vTb, rcol, ccol = per_batch[b]
        in_maps.append({
            "qT": qTb, "kT": kTb, "vT": vTb,
            "wq": np.ascontiguousarray(np.asarray(Wq, np.float32)[hs, :].T).astype(bf16),
            "wk": np.ascontiguousarray(np.asarray(Wk, np.float32)[hs, :].T).astype(bf16),
            "wv": np.ascontiguousarray(np.asarray(Wv, np.float32)[hs, :].T).astype(bf16),
            "wo": np.ascontiguousarray(np.asarray(Wo, np.float32)[:, hs].T).astype(bf16),
            "rcol": rcol, "ccol": ccol,
        })
    return in_maps


def kernel(q, k, v, reaches, Wq, Wk, Wv, Wo, **run_kwargs):
    nc = build_nc()
    in_maps = _prep_inputs(q, k, v, reaches, Wq, Wk, Wv, Wo)
    res = run_bass_kernel_spmd(nc, in_maps, list(range(8)), **run_kwargs)
    out = np.zeros((B, S, D), np.float32)
    for c in range(8):
        b = c // 4
        out[b] += np.asarray(res.results[c]["y"], dtype=np.float32)
    kernel.last_results = res
    return out
